# revision 17
# baseline (speedup 1.0000x reference)
"""Trainium2 Bass kernel for nn_ChargeSpinEmbedding.

Computation (per atom n with graph g = batch_segments[n]):
    q = onehot @ Wq.T ; k,v = W{k,v}[psi_g < 0]
    y = softplus((q.k)/sqrt(F)) ; att = psi_g * y / (segsum(y) + eps)
    v_att = att * v ; out = v_att + silu(silu(v_att) @ W1.T) @ W2.T

Device algorithm (algebraically identical):
    A_b = onehot @ u_b,  u_b = Wq.T @ Wk[b]   # per-tile matmuls, fp16 inputs
    y_b = softplus(A_b / sqrt(F))             # both branches, select later
    denom_b = segment_sum(y_b)                # cumsum + gather at segment ends
    f = psi / (denom_sel + eps) ; f0 = f*(psi>=0) ; f1 = f*(psi<0)
    C = [f0[g]*y0 ; f1[g]*y1]                 # branch masks pick the right y
    v_att.T = Wv.T @ C                        # feature-major
    out = C.T@Wv + silu(silu(v_att)@W1.T).T @ W2.T   # atom-major psum accum

Sharding: atoms split across 8 cores at graph boundaries (whole graphs live
on one core).  Within a core, atoms are laid out in 8 stripes of GRP=4224
(33 tiles of 128) along the free dim, one stripe per 16-partition group
(GpSimd core granularity for ap_gather).  Per-atom scalar arrays are
[128, GRP] with rows 16g..16g+15 alternating branch0/branch1 values.
"""

import contextlib
import numpy as np

import concourse.bacc as bacc
import concourse.tile as tile
from concourse import mybir
from concourse.bass_utils import run_bass_kernel_spmd

F32 = mybir.dt.float32
F16 = mybir.dt.float16
I16 = mybir.dt.int16
ALU = mybir.AluOpType
ACTF = mybir.ActivationFunctionType

N_CORES = 8
E = 118            # num elements
F = 128            # num features
NGRP = 8           # gpsimd groups (16 partitions each)
TPG = 33           # 128-atom tiles per group
GRP = TPG * 128    # atoms per group along free dim = 4224
NC_PAD = NGRP * GRP  # padded atoms per core = 33792
GTAB = 1152        # padded graphs per core
CS_W = GRP + 4     # csum row width; col GRP is the zero slot
SCALE = float(1.0 / np.sqrt(F))
EPS = 1e-6
P3W = [1024, 1024, 1024, 1024, 128]   # phase-3 chunk widths per group

_cached = {}
TRACE = False          # set True to capture an NTFF profile
LAST_EXEC_NS = None    # exec_time_ns of the last run when TRACE


def _build_program():
    nc = bacc.Bacc("TRN2", target_bir_lowering=False, debug=False,
                   num_devices=N_CORES)

    # one-hot rows host-padded from E=118 to 128 floats so the fp16
    # transpose blocks are [128,128] (xbar needs free%128==0)
    oh_in = nc.dram_tensor("oh_in", [NC_PAD, 128], F32, kind="ExternalInput").ap()
    psi_in = nc.dram_tensor("psi_in", [1, GTAB], F32, kind="ExternalInput").ap()
    gidA_in = nc.dram_tensor("gidA_in", [128, GRP // 16], I16, kind="ExternalInput").ap()
    gidB_in = nc.dram_tensor("gidB_in", [128, GRP // 16], I16, kind="ExternalInput").ap()
    endp_in = nc.dram_tensor("endp_in", [128, GTAB // 16], I16, kind="ExternalInput").ap()
    stap_in = nc.dram_tensor("stap_in", [128, GTAB // 16], I16, kind="ExternalInput").ap()
    sel_in = nc.dram_tensor("sel_in", [128, 2], F32, kind="ExternalInput").ap()
    ident_in = nc.dram_tensor("ident_in", [128, 128], F32, kind="ExternalInput").ap()
    wq_in = nc.dram_tensor("wq_in", [F, E], F32, kind="ExternalInput").ap()
    wk_in = nc.dram_tensor("wk_in", [2, F], F32, kind="ExternalInput").ap()
    wv_in = nc.dram_tensor("wv_in", [2, F], F32, kind="ExternalInput").ap()
    w1_in = nc.dram_tensor("w1_in", [F, F], F32, kind="ExternalInput").ap()
    w2_in = nc.dram_tensor("w2_in", [F, F], F32, kind="ExternalInput").ap()
    out_dram = nc.dram_tensor("out", [NC_PAD, F], F32, kind="ExternalOutput").ap()

    with tile.TileContext(nc) as tc, contextlib.ExitStack() as ctx:
        const = ctx.enter_context(tc.tile_pool(name="const", bufs=1))
        big = ctx.enter_context(tc.tile_pool(name="big", bufs=1))

        # ---- params ----
        wq_s = const.tile([F, E], F32)
        nc.sync.dma_start(wq_s[:], wq_in)
        wv_s = const.tile([2, F], F32)
        nc.sync.dma_start(wv_s[:], wv_in)
        w1_s = const.tile([F, F], F32)
        nc.sync.dma_start(w1_s[:], w1_in)
        w2_s = const.tile([F, F], F32)
        nc.sync.dma_start(w2_s[:], w2_in)
        ident_s = const.tile([128, 128], F32)
        nc.sync.dma_start(ident_s[:], ident_in)
        sel_s = const.tile([128, 2], F32)
        nc.sync.dma_start(sel_s[:], sel_in)
        psi_s = const.tile([1, GTAB], F32)
        nc.sync.dma_start(psi_s[:], psi_in)
        gidA_s = const.tile([128, GRP // 16], I16)
        nc.sync.dma_start(gidA_s[:], gidA_in)
        gidB_s = const.tile([128, GRP // 16], I16)
        nc.sync.dma_start(gidB_s[:], gidB_in)
        endp_s = const.tile([128, GTAB // 16], I16)
        nc.sync.dma_start(endp_s[:], endp_in)
        stap_s = const.tile([128, GTAB // 16], I16)
        nc.sync.dma_start(stap_s[:], stap_in)
        wkT_s = const.tile([F, 2], F32)
        nc.sync.dma_start(wkT_s[:], wk_in.rearrange("a b -> b a"))

        # ---- derived params: u16 = fp16(Wq.T @ Wk.T), W1T, W2T ----
        u16 = const.tile([E, 2], F16)
        w1t_s = const.tile([F, F], F32)
        w2t_s = const.tile([F, F], F32)
        with tc.tile_pool(name="setup_ps", bufs=1, space="PSUM") as sps:
            u_ps = sps.tile([E, 2], F32)
            nc.tensor.matmul(u_ps[:], lhsT=wq_s[:], rhs=wkT_s[:],
                             start=True, stop=True)
            nc.scalar.activation(u16[:], u_ps[:], ACTF.Copy)
            w1t_ps = sps.tile([F, F], F32)
            nc.tensor.transpose(w1t_ps[:], w1_s[:], ident_s[:])
            nc.scalar.activation(w1t_s[:], w1t_ps[:], ACTF.Copy)
            w2t_ps = sps.tile([F, F], F32)
            nc.tensor.transpose(w2t_ps[:], w2_s[:], ident_s[:])
            nc.scalar.activation(w2t_s[:], w2t_ps[:], ACTF.Copy)

        # ---- phase 1: A (tile-major) -> softplus -> y01 flat fp16 ----
        # y01[16g+r, 128*t+p] = y_{r%2}(atom g*GRP + 128*t + p)
        y01 = big.tile([128, GRP], F16)
        with tc.tile_pool(name="p1_sbuf", bufs=3) as p1, \
                tc.tile_pool(name="p1_y", bufs=2) as p1y, \
                tc.tile_pool(name="p1_ps", bufs=2, space="PSUM") as p1ps:
            for g in range(NGRP):
                # a01 columns: [0:TPG) = branch 0 by tile, [TPG:2TPG) = branch 1
                a01_ps = p1ps.tile([128, 2 * TPG], F32, tag="a01")
                a01v = a01_ps.rearrange("p (two t) -> p t two", two=2)
                oT16 = p1y.tile([128, GRP], F16, tag="oT")
                for tl0 in range(0, TPG, 4):
                    bw = min(4, TPG - tl0)
                    w = bw * 128
                    a_off = g * GRP + tl0 * 128
                    oh_t = p1.tile([128, bw * 128], F32, tag="oh")
                    nc.sync.dma_start(
                        oh_t.rearrange("p (t e) -> p t e", e=128),
                        oh_in[a_off:a_off + w, :].rearrange(
                            "(t p) e -> p t e", p=128))
                    oh16_t = p1.tile([128, bw * 128], F16, tag="oh16")
                    nc.gpsimd.tensor_copy(oh16_t[:], oh_t[:])
                    for t in range(bw):
                        tl = tl0 + t
                        nc.sync.dma_start_transpose(
                            oT16[:, tl * 128:(tl + 1) * 128],
                            oh16_t[:, t * 128:(t + 1) * 128])
                for tl in range(TPG):
                    nc.tensor.matmul(
                        a01v[:, tl:tl + 1, :],
                        lhsT=oT16[0:E, tl * 128:(tl + 1) * 128],
                        rhs=u16[:], start=True, stop=True)
                # softplus(A*s) = max(A,0)*s + ln(1 + exp(-|A|*s))
                # (hw has no softplus table; Exp and Ln share one set)
                absx = p1y.tile([128, 2 * TPG], F32, tag="absx")
                nc.scalar.activation(absx[:], a01_ps[:], ACTF.Abs,
                                     scale=SCALE)
                ex = p1y.tile([128, 2 * TPG], F32, tag="ex")
                nc.scalar.activation(ex[:], absx[:], ACTF.Exp, scale=-1.0)
                lg = p1y.tile([128, 2 * TPG], F32, tag="lg")
                nc.scalar.activation(lg[:], ex[:], ACTF.Ln, bias=1.0)
                rx = p1y.tile([128, 2 * TPG], F32, tag="rx")
                nc.vector.tensor_scalar(rx[:], a01_ps[:], 0.0, SCALE,
                                        ALU.max, ALU.mult)
                ytile = p1y.tile([128, 128], F16, tag="ytile")
                nc.vector.memset(ytile[:, 2 * TPG:128], 0.0)
                nc.vector.tensor_tensor(ytile[:, 0:2 * TPG], rx[:], lg[:],
                                        ALU.add)
                yT = p1y.tile([128, 128], F16, tag="yT")
                nc.sync.dma_start_transpose(yT[:], ytile[:])
                # yT rows [0:TPG) = branch0 tiles, [TPG:2TPG) = branch1
                for b in range(2):
                    nc.sync.dma_start(
                        y01[16 * g + b:16 * g + b + 1, :].rearrange(
                            "p (t f) -> p t f", f=128),
                        yT[b * TPG:(b + 1) * TPG, :].rearrange(
                            "t (one f) -> t one f", one=1))
                for r in range(2, 16):
                    nc.sync.dma_start(y01[16 * g + r:16 * g + r + 1, :],
                                      y01[16 * g + (r % 2):16 * g + (r % 2) + 1, :])

        # ---- phase 2: cumsum, per-graph denom, f tables, expand ----
        cs = big.tile([128, CS_W], F32)
        nc.vector.memset(cs[:, GRP:CS_W], 0.0)
        nc.vector.tensor_tensor_scan(cs[:, 0:GRP], y01[:], y01[:], 0.0,
                                     ALU.add, ALU.bypass)
        fyA = big.tile([128, GRP], F32)
        pm = big.tile([128, GRP], F32)
        with tc.tile_pool(name="p2_sbuf", bufs=1) as p2, \
                tc.tile_pool(name="p2_ps", bufs=1, space="PSUM") as p2ps:
            ge = p2.tile([128, GTAB], F32)
            nc.gpsimd.ap_gather(ge[:], cs[:], endp_s[:], channels=128,
                                num_elems=CS_W, d=1, num_idxs=GTAB)
            gs = p2.tile([128, GTAB], F32)
            nc.gpsimd.ap_gather(gs[:], cs[:], stap_s[:], channels=128,
                                num_elems=CS_W, d=1, num_idxs=GTAB)
            p01 = p2.tile([128, GTAB], F32)
            nc.vector.tensor_tensor(p01[:], ge[:], gs[:], ALU.subtract)
            den_ps = p2ps.tile([2, 1536], F32)
            for c0 in range(0, GTAB, 512):
                c1 = min(c0 + 512, GTAB)
                nc.tensor.matmul(den_ps[:, c0:c1], lhsT=sel_s[:],
                                 rhs=p01[:, c0:c1], start=True, stop=True)
            den01 = p2.tile([2, GTAB], F32)
            nc.scalar.activation(den01[:], den_ps[:, 0:GTAB], ACTF.Copy)
            den0r = den01[0:1, :]
            den1r = p2.tile([1, GTAB], F32)
            nc.sync.dma_start(den1r[:], den01[1:2, :])
            bm = p2.tile([1, GTAB], F32)
            nc.vector.tensor_single_scalar(bm[:], psi_s[:], 0.0, ALU.is_lt)
            # den_sel = den0 + bm*(den1-den0); f = 8*psi/(den_sel + 8*eps)
            nc.vector.tensor_tensor(den1r[:], den1r[:], den0r, ALU.subtract)
            nc.vector.tensor_tensor(den1r[:], den1r[:], bm[:], ALU.mult)
            dsel = p2.tile([1, GTAB], F32)
            nc.vector.tensor_tensor(dsel[:], den0r, den1r[:], ALU.add)
            nc.vector.tensor_single_scalar(dsel[:], dsel[:],
                                           float(NGRP * EPS), ALU.add)
            psi8 = p2.tile([1, GTAB], F32)
            nc.vector.tensor_single_scalar(psi8[:], psi_s[:], float(NGRP),
                                           ALU.mult)
            drec = p2.tile([1, GTAB], F32)
            nc.vector.reciprocal(drec[:], dsel[:])
            f_t = p2.tile([1, GTAB], F32)
            nc.vector.tensor_tensor(f_t[:], psi8[:], drec[:], ALU.mult)
            ftr = p2.tile([1, 2 * GTAB], F32)
            nc.vector.tensor_tensor(ftr[:, GTAB:2 * GTAB], f_t[:], bm[:],
                                    ALU.mult)
            nc.vector.tensor_tensor(ftr[:, 0:GTAB], f_t[:],
                                    ftr[:, GTAB:2 * GTAB], ALU.subtract)
            ftab = p2.tile([128, 2 * GTAB], F32)
            nc.gpsimd.partition_broadcast(ftab[:], ftr[:], channels=128)
            # expand per-graph f to per-atom; odd rows take branch 1
            nc.gpsimd.ap_gather(fyA[:], ftab[:], gidA_s[:], channels=128,
                                num_elems=2 * GTAB, d=1, num_idxs=GRP)
            fyB = p2.tile([128, GRP], F32)
            nc.gpsimd.ap_gather(fyB[:], ftab[:], gidB_s[:], channels=128,
                                num_elems=2 * GTAB, d=1, num_idxs=GRP)
            for g in range(NGRP):
                nc.sync.dma_start(fyA[16 * g + 1:16 * g + 2, :],
                                  fyB[16 * g + 1:16 * g + 2, :])
            nc.vector.tensor_tensor(pm[:], fyA[:], y01[:], ALU.mult)

        # ---- phase 3: v_att, MLP, out (atom-major psum accumulation) ----
        with tc.tile_pool(name="p3_sbuf", bufs=2) as p3, \
                tc.tile_pool(name="p3_out", bufs=3) as p3o, \
                tc.tile_pool(name="p3_vps", bufs=1, space="PSUM") as vps, \
                tc.tile_pool(name="p3_hps", bufs=1, space="PSUM") as hps, \
                tc.tile_pool(name="p3_ops", bufs=1, space="PSUM") as ops:
            for g in range(NGRP):
                col = 0
                for w in P3W:
                    a_off = g * GRP + col
                    # PE operands must be based at partition 0/32/64: stage
                    # the two C rows of this group into partitions 0-1
                    cst = p3.tile([2, w], F32, tag="cst")
                    nc.sync.dma_start(cst[0:1, :],
                                      pm[16 * g:16 * g + 1, col:col + w])
                    nc.sync.dma_start(cst[1:2, :],
                                      pm[16 * g + 1:16 * g + 2, col:col + w])
                    vatt_ps = vps.tile([128, w], F32, tag="vatt")
                    for c0 in range(0, w, 512):
                        cw = min(512, w - c0)
                        nc.tensor.matmul(vatt_ps[:, c0:c0 + cw],
                                         lhsT=wv_s[:], rhs=cst[:, c0:c0 + cw],
                                         start=True, stop=True)
                    sv = p3.tile([128, w], F32, tag="sv")
                    nc.scalar.activation(sv[:], vatt_ps[:], ACTF.Silu)
                    h_ps = hps.tile([128, w], F32, tag="h")
                    for c0 in range(0, w, 512):
                        cw = min(512, w - c0)
                        nc.tensor.matmul(h_ps[:, c0:c0 + cw], lhsT=w1t_s[:],
                                         rhs=sv[:, c0:c0 + cw],
                                         start=True, stop=True)
                    sh = p3.tile([128, w], F32, tag="sh")
                    nc.scalar.activation(sh[:], h_ps[:], ACTF.Silu)
                    out_ps = ops.tile([128, w], F32, tag="out")
                    nt = w // 128
                    for t in range(nt):
                        nc.tensor.matmul(
                            out_ps[:, t * 128:(t + 1) * 128],
                            lhsT=cst[:, t * 128:(t + 1) * 128],
                            rhs=wv_s[:], start=True, stop=False)
                        nc.tensor.matmul(
                            out_ps[:, t * 128:(t + 1) * 128],
                            lhsT=sh[:, t * 128:(t + 1) * 128],
                            rhs=w2t_s[:], start=False, stop=True)
                    out_sb = p3o.tile([128, w], F32, tag="osb")
                    nc.vector.tensor_copy(out_sb[:], out_ps[:])
                    nc.sync.dma_start(
                        out_dram[a_off:a_off + w, :].rearrange(
                            "(t p) f -> p t f", p=128),
                        out_sb.rearrange("p (t f) -> p t f", f=F))
                    col += w

    nc.compile()
    return nc


def _get_program():
    if "nc" not in _cached:
        _cached["nc"] = _build_program()
    return _cached["nc"]


def _prepare_core_inputs(oh, psi, bs, Wq, Wk, Wv, W1, W2):
    """Host-side sharding + integer metadata construction."""
    N = oh.shape[0]
    bounds = [0]
    for c in range(1, N_CORES):
        g = bs[(N * c) // N_CORES]
        bounds.append(int(np.searchsorted(bs, g, side="left")))
    bounds.append(N)

    sel = np.zeros((128, 2), np.float32)
    sel[0::2, 0] = 1.0
    sel[1::2, 1] = 1.0
    ident = np.eye(128, dtype=np.float32)

    def wrap16(a):
        # a: [NGRP, L] -> [128, L//16] in ap_gather wrapped layout
        L = a.shape[1]
        outw = np.zeros((128, L // 16), a.dtype)
        for g in range(NGRP):
            outw[16 * g:16 * (g + 1), :] = a[g].reshape(L // 16, 16).T
        return outw

    in_maps = []
    meta = []
    for c in range(N_CORES):
        a0, a1 = bounds[c], bounds[c + 1]
        nca = a1 - a0
        assert nca <= NC_PAD, f"core {c} has {nca} atoms > NC_PAD"
        bs_c = bs[a0:a1]
        g0, g1 = int(bs_c[0]), int(bs_c[-1]) + 1
        gtab = g1 - g0
        assert gtab < GTAB, f"core {c} has {gtab} graphs >= GTAB"

        oh_c = np.zeros((NC_PAD, 128), np.float32)
        oh_c[:nca, :E] = oh[a0:a1]
        psi_c = np.zeros((1, GTAB), np.float32)
        psi_c[0, :gtab] = psi[g0:g1]

        gid = np.full(NC_PAD, gtab, np.int16)  # pad atoms -> pad graph
        gid[:nca] = (bs_c - g0).astype(np.int16)
        gidA = wrap16(gid.reshape(NGRP, GRP))
        gidB = gidA + np.int16(GTAB)

        gids = np.arange(g0, g1)
        starts = np.searchsorted(bs_c, gids, side="left")
        ends = np.searchsorted(bs_c, gids, side="right")
        endp = np.full((NGRP, GTAB), GRP, np.int16)   # zero slot
        stap = np.full((NGRP, GTAB), GRP, np.int16)
        for g in range(NGRP):
            lo, hi = g * GRP, (g + 1) * GRP
            s_ = np.clip(starts, lo, hi)
            e_ = np.clip(ends, lo, hi)
            has = e_ > s_
            endp[g, :gtab] = np.where(has, e_ - 1 - lo, GRP).astype(np.int16)
            stap[g, :gtab] = np.where(has & (s_ > lo), s_ - 1 - lo,
                                      GRP).astype(np.int16)
        in_maps.append({
            "oh_in": oh_c,
            "psi_in": psi_c,
            "gidA_in": gidA,
            "gidB_in": gidB,
            "endp_in": wrap16(endp),
            "stap_in": wrap16(stap),
            "sel_in": sel,
            "ident_in": ident,
            "wq_in": Wq,
            "wk_in": Wk,
            "wv_in": Wv,
            "w1_in": W1,
            "w2_in": W2,
        })
        meta.append((a0, a1))
    return in_maps, meta


def kernel(elements_one_hot, psi, Wq, Wk, Wv, W1, W2, batch_segments,
           num_graphs):
    oh = np.ascontiguousarray(np.asarray(elements_one_hot, np.float32))
    psi = np.ascontiguousarray(np.asarray(psi, np.float32))
    bs = np.ascontiguousarray(np.asarray(batch_segments, np.int64))
    Wq_ = np.ascontiguousarray(np.asarray(Wq, np.float32))
    Wk_ = np.ascontiguousarray(np.asarray(Wk, np.float32))
    Wv_ = np.ascontiguousarray(np.asarray(Wv, np.float32))
    W1_ = np.ascontiguousarray(np.asarray(W1, np.float32))
    W2_ = np.ascontiguousarray(np.asarray(W2, np.float32))

    in_maps, meta = _prepare_core_inputs(oh, psi, bs, Wq_, Wk_, Wv_, W1_, W2_)
    nc = _get_program()
    global LAST_EXEC_NS
    try:
        res = run_bass_kernel_spmd(nc, in_maps, list(range(N_CORES)),
                                   trace=TRACE)
        LAST_EXEC_NS = res.exec_time_ns
    except ModuleNotFoundError:
        res = run_bass_kernel_spmd(nc, in_maps, list(range(N_CORES)))
        LAST_EXEC_NS = None
    out = np.zeros((oh.shape[0], F), np.float32)
    for c, (a0, a1) in enumerate(meta):
        out[a0:a1] = np.asarray(res.results[c]["out"])[:a1 - a0]
    return out


# revision 28
# speedup vs baseline: 1.2683x; 1.2683x over previous
"""Trainium2 Bass kernel for nn_ChargeSpinEmbedding.

Computation (per atom n with graph g = batch_segments[n]):
    q = onehot @ Wq.T ; k,v = W{k,v}[psi_g < 0]
    y = softplus((q.k)/sqrt(F)) ; att = psi_g * y / (segsum(y) + eps)
    v_att = att * v ; out = v_att + silu(silu(v_att) @ W1.T) @ W2.T

Device algorithm (algebraically identical):
    A_b = onehot @ u_b,  u_b = Wq.T @ Wk[b]   # per-tile matmuls, fp16 inputs
    y_b = softplus(A_b / sqrt(F))             # both branches, select later
    denom_b = segment_sum(y_b)                # cumsum + gather at segment ends
    f = psi / (denom_sel + eps) ; f0 = f*(psi>=0) ; f1 = f*(psi<0)
    C = [f0[g]*y0 ; f1[g]*y1]                 # branch masks pick the right y
    v_att.T = Wv.T @ C                        # feature-major
    out = C.T@Wv + silu(silu(v_att)@W1.T).T @ W2.T   # atom-major psum accum

Sharding: atoms split across 8 cores at graph boundaries (whole graphs live
on one core).  Within a core, atoms are laid out in 8 stripes of GRP=4224
(33 tiles of 128) along the free dim, one stripe per 16-partition group
(GpSimd core granularity for ap_gather).  Per-atom scalar arrays are
[128, GRP] with rows 16g..16g+15 alternating branch0/branch1 values.
"""

import contextlib
import numpy as np

import concourse.bacc as bacc
import concourse.tile as tile
from concourse import mybir
from concourse.bass_utils import run_bass_kernel_spmd

F32 = mybir.dt.float32
F16 = mybir.dt.float16
I16 = mybir.dt.int16
ALU = mybir.AluOpType
ACTF = mybir.ActivationFunctionType

N_CORES = 8
E = 118            # num elements
F = 128            # num features
NGRP = 8           # gpsimd groups (16 partitions each)
TPG = 33           # 128-atom tiles per group
GRP = TPG * 128    # atoms per group along free dim = 4224
NC_PAD = NGRP * GRP  # padded atoms per core = 33792
GTAB = 1152        # padded graphs per core
CS_W = GRP + 4     # csum row width; col GRP is the zero slot
SCALE = float(1.0 / np.sqrt(F))
EPS = 1e-6
P3W = [1024, 1024, 1024, 1024, 128]   # phase-3 chunk widths per group

_cached = {}
TRACE = False          # set True to capture an NTFF profile
LAST_EXEC_NS = None    # exec_time_ns of the last run when TRACE


def _build_program():
    nc = bacc.Bacc("TRN2", target_bir_lowering=False, debug=False,
                   num_devices=N_CORES)

    # one-hot host-transposed to [128, NC_PAD] fp16 (exact for 0/1 data):
    # feature-major loads are one big contiguous DMA per group and feed the
    # A-matmuls directly (no on-device transpose or convert)
    oh_in = nc.dram_tensor("oh_in", [128, NC_PAD], F16, kind="ExternalInput").ap()
    psi_in = nc.dram_tensor("psi_in", [1, GTAB], F32, kind="ExternalInput").ap()
    gidA_in = nc.dram_tensor("gidA_in", [128, GRP // 16], I16, kind="ExternalInput").ap()
    gidB_in = nc.dram_tensor("gidB_in", [128, GRP // 16], I16, kind="ExternalInput").ap()
    endp_in = nc.dram_tensor("endp_in", [128, GTAB // 16], I16, kind="ExternalInput").ap()
    stap_in = nc.dram_tensor("stap_in", [128, GTAB // 16], I16, kind="ExternalInput").ap()
    sel_in = nc.dram_tensor("sel_in", [128, 2], F32, kind="ExternalInput").ap()
    ident_in = nc.dram_tensor("ident_in", [128, 128], F32, kind="ExternalInput").ap()
    wq_in = nc.dram_tensor("wq_in", [F, E], F32, kind="ExternalInput").ap()
    wk_in = nc.dram_tensor("wk_in", [2, F], F32, kind="ExternalInput").ap()
    wv_in = nc.dram_tensor("wv_in", [2, F], F32, kind="ExternalInput").ap()
    w1_in = nc.dram_tensor("w1_in", [F, F], F32, kind="ExternalInput").ap()
    w2_in = nc.dram_tensor("w2_in", [F, F], F32, kind="ExternalInput").ap()
    out_dram = nc.dram_tensor("out", [NC_PAD, F], F32, kind="ExternalOutput").ap()

    with tile.TileContext(nc) as tc, contextlib.ExitStack() as ctx:
        const = ctx.enter_context(tc.tile_pool(name="const", bufs=1))
        big = ctx.enter_context(tc.tile_pool(name="big", bufs=1))

        # ---- params ----
        wq_s = const.tile([F, E], F32)
        nc.sync.dma_start(wq_s[:], wq_in)
        wv_s = const.tile([2, F], F32)
        nc.sync.dma_start(wv_s[:], wv_in)
        w1_s = const.tile([F, F], F32)
        nc.sync.dma_start(w1_s[:], w1_in)
        w2_s = const.tile([F, F], F32)
        nc.sync.dma_start(w2_s[:], w2_in)
        ident_s = const.tile([128, 128], F32)
        nc.sync.dma_start(ident_s[:], ident_in)
        sel_s = const.tile([128, 2], F32)
        nc.sync.dma_start(sel_s[:], sel_in)
        psi_s = const.tile([1, GTAB], F32)
        nc.sync.dma_start(psi_s[:], psi_in)
        gidA_s = const.tile([128, GRP // 16], I16)
        nc.sync.dma_start(gidA_s[:], gidA_in)
        gidB_s = const.tile([128, GRP // 16], I16)
        nc.sync.dma_start(gidB_s[:], gidB_in)
        endp_s = const.tile([128, GTAB // 16], I16)
        nc.sync.dma_start(endp_s[:], endp_in)
        stap_s = const.tile([128, GTAB // 16], I16)
        nc.sync.dma_start(stap_s[:], stap_in)
        wkT_s = const.tile([F, 2], F32)
        nc.sync.dma_start(wkT_s[:], wk_in.rearrange("a b -> b a"))

        # ---- derived params: u16 = fp16(Wq.T @ Wk.T), W1T, W2T ----
        u16 = const.tile([E, 2], F16)
        w1t_s = const.tile([F, F], F32)
        w2t_s = const.tile([F, F], F32)
        with tc.tile_pool(name="setup_ps", bufs=1, space="PSUM") as sps:
            u_ps = sps.tile([E, 2], F32)
            nc.tensor.matmul(u_ps[:], lhsT=wq_s[:], rhs=wkT_s[:],
                             start=True, stop=True)
            nc.scalar.activation(u16[:], u_ps[:], ACTF.Copy)
            w1t_ps = sps.tile([F, F], F32)
            nc.tensor.transpose(w1t_ps[:], w1_s[:], ident_s[:])
            nc.scalar.activation(w1t_s[:], w1t_ps[:], ACTF.Copy)
            w2t_ps = sps.tile([F, F], F32)
            nc.tensor.transpose(w2t_ps[:], w2_s[:], ident_s[:])
            nc.scalar.activation(w2t_s[:], w2t_ps[:], ACTF.Copy)

        # ---- phase 1: A (tile-major) -> softplus -> y01 flat fp16 ----
        # y01[16g+r, 128*t+p] = y_{r%2}(atom g*GRP + 128*t + p)
        y01 = big.tile([128, GRP], F16)
        _cached.setdefault("dbg", {})["y01"] = y01.tensor.name
        with tc.tile_pool(name="p1_sbuf", bufs=3) as p1, \
                tc.tile_pool(name="p1_y", bufs=2) as p1y, \
                tc.tile_pool(name="p1_ps", bufs=2, space="PSUM") as p1ps:
            for g in range(NGRP):
                # a01 columns: [0:TPG) = branch 0 by tile, [TPG:2TPG) = branch 1
                a01_ps = p1ps.tile([128, 2 * TPG], F32, tag="a01")
                a01v = a01_ps.rearrange("p (two t) -> p t two", two=2)
                oT16 = p1y.tile([128, GRP], F16, tag="oT")
                nc.sync.dma_start(oT16[:], oh_in[:, g * GRP:(g + 1) * GRP])
                for tl in range(TPG):
                    nc.tensor.matmul(
                        a01v[:, tl:tl + 1, :],
                        lhsT=oT16[0:E, tl * 128:(tl + 1) * 128],
                        rhs=u16[:], start=True, stop=True)
                # softplus(A*s) = max(A,0)*s + ln(1 + exp(-|A|*s))
                # (hw has no softplus table; Exp and Ln share one set)
                absx = p1y.tile([128, 2 * TPG], F32, tag="absx")
                nc.scalar.activation(absx[:], a01_ps[:], ACTF.Abs,
                                     scale=SCALE)
                ex = p1y.tile([128, 2 * TPG], F32, tag="ex")
                nc.scalar.activation(ex[:], absx[:], ACTF.Exp, scale=-1.0)
                lg = p1y.tile([128, 2 * TPG], F32, tag="lg")
                nc.scalar.activation(lg[:], ex[:], ACTF.Ln, bias=1.0)
                rx = p1y.tile([128, 2 * TPG], F32, tag="rx")
                nc.vector.tensor_scalar(rx[:], a01_ps[:], 0.0, SCALE,
                                        ALU.max, ALU.mult)
                ytile = p1y.tile([128, 128], F16, tag="ytile")
                nc.vector.memset(ytile[:, 2 * TPG:128], 0.0)
                nc.vector.tensor_tensor(ytile[:, 0:2 * TPG], rx[:], lg[:],
                                        ALU.add)
                yT = p1y.tile([128, 128], F16, tag="yT")
                nc.sync.dma_start_transpose(yT[:], ytile[:])
                # yT rows [0:TPG) = branch0 tiles, [TPG:2TPG) = branch1.
                # y01 rows per group: [16g:16g+8) = branch0, [16g+8:16g+16) = b1
                for b in range(2):
                    r0 = 16 * g + 8 * b
                    nc.sync.dma_start(
                        y01[r0:r0 + 1, :].rearrange("p (t f) -> p t f", f=128),
                        yT[b * TPG:(b + 1) * TPG, :].rearrange(
                            "t (one f) -> t one f", one=1))
                    nc.sync.dma_start(
                        y01[r0 + 1:r0 + 8, :],
                        y01[r0:r0 + 1, None, :].to_broadcast((1, 7, GRP)))

        # ---- phase 2: cumsum, per-graph denom, f tables, expand ----
        cs = big.tile([128, CS_W], F32)
        _cached.setdefault("dbg", {})["cs"] = cs.tensor.name
        nc.vector.memset(cs[:, GRP:CS_W], 0.0)
        nc.vector.tensor_tensor_scan(cs[:, 0:GRP], y01[:], y01[:], 0.0,
                                     ALU.add, ALU.bypass)
        fyA = big.tile([128, GRP], F32)
        _cached.setdefault("dbg", {})["fyA"] = fyA.tensor.name
        pm = big.tile([128, GRP], F32)
        _cached.setdefault("dbg", {})["pm"] = pm.tensor.name
        with tc.tile_pool(name="p2_sbuf", bufs=1) as p2, \
                tc.tile_pool(name="p2_ps", bufs=1, space="PSUM") as p2ps:
            ge = p2.tile([128, GTAB], F32)
            nc.gpsimd.ap_gather(ge[:], cs[:], endp_s[:], channels=128,
                                num_elems=CS_W, d=1, num_idxs=GTAB)
            gs = p2.tile([128, GTAB], F32)
            nc.gpsimd.ap_gather(gs[:], cs[:], stap_s[:], channels=128,
                                num_elems=CS_W, d=1, num_idxs=GTAB)
            p01 = p2.tile([128, GTAB], F32)
            nc.vector.tensor_tensor(p01[:], ge[:], gs[:], ALU.subtract)
            den_ps = p2ps.tile([2, 1536], F32)
            for c0 in range(0, GTAB, 512):
                c1 = min(c0 + 512, GTAB)
                nc.tensor.matmul(den_ps[:, c0:c1], lhsT=sel_s[:],
                                 rhs=p01[:, c0:c1], start=True, stop=True)
            den01 = p2.tile([2, GTAB], F32)
            nc.scalar.activation(den01[:], den_ps[:, 0:GTAB], ACTF.Copy)
            den0r = den01[0:1, :]
            den1r = p2.tile([1, GTAB], F32)
            nc.sync.dma_start(den1r[:], den01[1:2, :])
            bm = p2.tile([1, GTAB], F32)
            nc.vector.tensor_single_scalar(bm[:], psi_s[:], 0.0, ALU.is_lt)
            # den_sel = den0 + bm*(den1-den0); f = 8*psi/(den_sel + 8*eps)
            nc.vector.tensor_tensor(den1r[:], den1r[:], den0r, ALU.subtract)
            nc.vector.tensor_tensor(den1r[:], den1r[:], bm[:], ALU.mult)
            dsel = p2.tile([1, GTAB], F32)
            nc.vector.tensor_tensor(dsel[:], den0r, den1r[:], ALU.add)
            nc.vector.tensor_single_scalar(dsel[:], dsel[:],
                                           float(NGRP * EPS), ALU.add)
            psi8 = p2.tile([1, GTAB], F32)
            nc.vector.tensor_single_scalar(psi8[:], psi_s[:], float(NGRP),
                                           ALU.mult)
            drec = p2.tile([1, GTAB], F32)
            nc.vector.reciprocal(drec[:], dsel[:])
            f_t = p2.tile([1, GTAB], F32)
            nc.vector.tensor_tensor(f_t[:], psi8[:], drec[:], ALU.mult)
            ftr = p2.tile([1, 2 * GTAB], F32)
            _cached.setdefault("dbg", {})["ftr"] = ftr.tensor.name
            nc.vector.tensor_tensor(ftr[:, GTAB:2 * GTAB], f_t[:], bm[:],
                                    ALU.mult)
            nc.vector.tensor_tensor(ftr[:, 0:GTAB], f_t[:],
                                    ftr[:, GTAB:2 * GTAB], ALU.subtract)
            ftab = p2.tile([128, 2 * GTAB], F32)
            nc.gpsimd.partition_broadcast(ftab[:], ftr[:], channels=128)
            # expand per-graph f to per-atom; odd rows take branch 1
            nc.gpsimd.ap_gather(fyA[:], ftab[:], gidA_s[:], channels=128,
                                num_elems=2 * GTAB, d=1, num_idxs=GRP)
            fyB = p2.tile([128, GRP], F32)
            nc.gpsimd.ap_gather(fyB[:], ftab[:], gidB_s[:], channels=128,
                                num_elems=2 * GTAB, d=1, num_idxs=GRP)
            for g in range(NGRP):
                nc.sync.dma_start(fyA[16 * g + 8:16 * g + 9, :],
                                  fyB[16 * g + 8:16 * g + 9, :])
            nc.vector.tensor_tensor(pm[:], fyA[:], y01[:], ALU.mult)

        # ---- phase 3: v_att, MLP, out (atom-major psum accumulation) ----
        with tc.tile_pool(name="p3_sbuf", bufs=2) as p3, \
                tc.tile_pool(name="p3_out", bufs=3) as p3o, \
                tc.tile_pool(name="p3_vps", bufs=1, space="PSUM") as vps, \
                tc.tile_pool(name="p3_hps", bufs=1, space="PSUM") as hps, \
                tc.tile_pool(name="p3_ops", bufs=1, space="PSUM") as ops:
            for g in range(NGRP):
                col = 0
                for w in P3W:
                    a_off = g * GRP + col
                    # PE operands must be based at partition 0/32/64: stage
                    # the two C rows of this group into partitions 0-1
                    cst = p3.tile([2, w], F32, tag="cst")
                    nc.sync.dma_start(cst[0:1, :],
                                      pm[16 * g:16 * g + 1, col:col + w])
                    nc.sync.dma_start(cst[1:2, :],
                                      pm[16 * g + 8:16 * g + 9, col:col + w])
                    vatt_ps = vps.tile([128, w], F32, tag="vatt")
                    for c0 in range(0, w, 512):
                        cw = min(512, w - c0)
                        nc.tensor.matmul(vatt_ps[:, c0:c0 + cw],
                                         lhsT=wv_s[:], rhs=cst[:, c0:c0 + cw],
                                         start=True, stop=True)
                    sv = p3.tile([128, w], F32, tag="sv")
                    nc.scalar.activation(sv[:], vatt_ps[:], ACTF.Silu)
                    h_ps = hps.tile([128, w], F32, tag="h")
                    for c0 in range(0, w, 512):
                        cw = min(512, w - c0)
                        nc.tensor.matmul(h_ps[:, c0:c0 + cw], lhsT=w1t_s[:],
                                         rhs=sv[:, c0:c0 + cw],
                                         start=True, stop=True)
                    sh = p3.tile([128, w], F32, tag="sh")
                    nc.scalar.activation(sh[:], h_ps[:], ACTF.Silu)
                    out_ps = ops.tile([128, w], F32, tag="out")
                    nt = w // 128
                    for t in range(nt):
                        nc.tensor.matmul(
                            out_ps[:, t * 128:(t + 1) * 128],
                            lhsT=cst[:, t * 128:(t + 1) * 128],
                            rhs=wv_s[:], start=True, stop=False)
                        nc.tensor.matmul(
                            out_ps[:, t * 128:(t + 1) * 128],
                            lhsT=sh[:, t * 128:(t + 1) * 128],
                            rhs=w2t_s[:], start=False, stop=True)
                    out_sb = p3o.tile([128, w], F32, tag="osb")
                    nc.vector.tensor_copy(out_sb[:], out_ps[:])
                    # store rows permuted (row 8p+t holds atom 128t+p) so
                    # each partition writes one contiguous 4KB run; the host
                    # un-permutes after download
                    nc.sync.dma_start(
                        out_dram[a_off:a_off + w, :].rearrange(
                            "(p t) f -> p t f", p=128),
                        out_sb.rearrange("p (t f) -> p t f", f=F))
                    col += w

    nc.compile()
    return nc


def _get_program():
    if "nc" not in _cached:
        _cached["nc"] = _build_program()
    return _cached["nc"]


def _prepare_core_inputs(oh, psi, bs, Wq, Wk, Wv, W1, W2):
    """Host-side sharding + integer metadata construction."""
    N = oh.shape[0]
    bounds = [0]
    for c in range(1, N_CORES):
        g = bs[(N * c) // N_CORES]
        bounds.append(int(np.searchsorted(bs, g, side="left")))
    bounds.append(N)

    sel = np.zeros((128, 2), np.float32)
    b1rows = (np.arange(128) % 16) >= 8
    sel[~b1rows, 0] = 1.0
    sel[b1rows, 1] = 1.0
    ident = np.eye(128, dtype=np.float32)

    def wrap16(a):
        # a: [NGRP, L] -> [128, L//16] in ap_gather wrapped layout
        L = a.shape[1]
        outw = np.zeros((128, L // 16), a.dtype)
        for g in range(NGRP):
            outw[16 * g:16 * (g + 1), :] = a[g].reshape(L // 16, 16).T
        return outw

    in_maps = []
    meta = []
    for c in range(N_CORES):
        a0, a1 = bounds[c], bounds[c + 1]
        nca = a1 - a0
        assert nca <= NC_PAD, f"core {c} has {nca} atoms > NC_PAD"
        bs_c = bs[a0:a1]
        g0, g1 = int(bs_c[0]), int(bs_c[-1]) + 1
        gtab = g1 - g0
        assert gtab < GTAB, f"core {c} has {gtab} graphs >= GTAB"

        oh_c = np.zeros((128, NC_PAD), np.float16)
        oh_c[:E, :nca] = oh[a0:a1].T  # fp16 is exact for one-hot 0/1 data
        psi_c = np.zeros((1, GTAB), np.float32)
        psi_c[0, :gtab] = psi[g0:g1]

        gid = np.full(NC_PAD, gtab, np.int16)  # pad atoms -> pad graph
        gid[:nca] = (bs_c - g0).astype(np.int16)
        gidA = wrap16(gid.reshape(NGRP, GRP))
        gidB = gidA + np.int16(GTAB)

        gids = np.arange(g0, g1)
        starts = np.searchsorted(bs_c, gids, side="left")
        ends = np.searchsorted(bs_c, gids, side="right")
        endp = np.full((NGRP, GTAB), GRP, np.int16)   # zero slot
        stap = np.full((NGRP, GTAB), GRP, np.int16)
        for g in range(NGRP):
            lo, hi = g * GRP, (g + 1) * GRP
            s_ = np.clip(starts, lo, hi)
            e_ = np.clip(ends, lo, hi)
            has = e_ > s_
            endp[g, :gtab] = np.where(has, e_ - 1 - lo, GRP).astype(np.int16)
            stap[g, :gtab] = np.where(has & (s_ > lo), s_ - 1 - lo,
                                      GRP).astype(np.int16)
        in_maps.append({
            "oh_in": oh_c,
            "psi_in": psi_c,
            "gidA_in": gidA,
            "gidB_in": gidB,
            "endp_in": wrap16(endp),
            "stap_in": wrap16(stap),
            "sel_in": sel,
            "ident_in": ident,
            "wq_in": Wq,
            "wk_in": Wk,
            "wv_in": Wv,
            "w1_in": W1,
            "w2_in": W2,
        })
        meta.append((a0, a1))
    return in_maps, meta


def kernel(elements_one_hot, psi, Wq, Wk, Wv, W1, W2, batch_segments,
           num_graphs):
    oh = np.ascontiguousarray(np.asarray(elements_one_hot, np.float32))
    psi = np.ascontiguousarray(np.asarray(psi, np.float32))
    bs = np.ascontiguousarray(np.asarray(batch_segments, np.int64))
    Wq_ = np.ascontiguousarray(np.asarray(Wq, np.float32))
    Wk_ = np.ascontiguousarray(np.asarray(Wk, np.float32))
    Wv_ = np.ascontiguousarray(np.asarray(Wv, np.float32))
    W1_ = np.ascontiguousarray(np.asarray(W1, np.float32))
    W2_ = np.ascontiguousarray(np.asarray(W2, np.float32))

    in_maps, meta = _prepare_core_inputs(oh, psi, bs, Wq_, Wk_, Wv_, W1_, W2_)
    nc = _get_program()
    global LAST_EXEC_NS
    try:
        res = run_bass_kernel_spmd(nc, in_maps, list(range(N_CORES)),
                                   trace=TRACE)
        LAST_EXEC_NS = res.exec_time_ns
    except ModuleNotFoundError:
        res = run_bass_kernel_spmd(nc, in_maps, list(range(N_CORES)))
        LAST_EXEC_NS = None
    out = np.zeros((oh.shape[0], F), np.float32)
    for c, (a0, a1) in enumerate(meta):
        dev = np.asarray(res.results[c]["out"])
        # undo the store permutation: per chunk, device row 8p+t is atom 128t+p
        unperm = np.empty_like(dev)
        for g in range(NGRP):
            col = 0
            for w in P3W:
                cb = g * GRP + col
                nt = w // 128
                blk = dev[cb:cb + w].reshape(128, nt, F).transpose(1, 0, 2)
                unperm[cb:cb + w] = blk.reshape(w, F)
                col += w
        out[a0:a1] = unperm[:a1 - a0]
    return out


# revision 30
# speedup vs baseline: 2.3171x; 1.8270x over previous
"""Trainium2 Bass kernel for nn_ChargeSpinEmbedding.

Computation (per atom n with graph g = batch_segments[n]):
    q = onehot @ Wq.T ; k,v = W{k,v}[psi_g < 0]
    y = softplus((q.k)/sqrt(F)) ; att = psi_g * y / (segsum(y) + eps)
    v_att = att * v ; out = v_att + silu(silu(v_att) @ W1.T) @ W2.T

Device algorithm (algebraically identical):
    A_b = onehot @ u_b,  u_b = Wq.T @ Wk[b]   # per-tile matmuls, fp16 inputs
    y_b = softplus(A_b / sqrt(F))             # both branches, select later
    denom_b = segment_sum(y_b)                # cumsum + gather at segment ends
    f = psi / (denom_sel + eps) ; f0 = f*(psi>=0) ; f1 = f*(psi<0)
    C = [f0[g]*y0 ; f1[g]*y1]                 # branch masks pick the right y
    v_att.T = Wv.T @ C                        # feature-major
    out = C.T@Wv + silu(silu(v_att)@W1.T).T @ W2.T   # atom-major psum accum

Sharding: atoms split across 8 cores at graph boundaries (whole graphs live
on one core).  Within a core, atoms are laid out in 8 stripes of GRP=4224
(33 tiles of 128) along the free dim, one stripe per 16-partition group
(GpSimd core granularity for ap_gather).  Per-atom scalar arrays are
[128, GRP] with rows 16g..16g+15 alternating branch0/branch1 values.
"""

import contextlib
import numpy as np

import concourse.bacc as bacc
import concourse.tile as tile
from concourse import mybir
from concourse.bass_utils import run_bass_kernel_spmd

F32 = mybir.dt.float32
F16 = mybir.dt.float16
I16 = mybir.dt.int16
ALU = mybir.AluOpType
ACTF = mybir.ActivationFunctionType

N_CORES = 8
E = 118            # num elements
F = 128            # num features
NGRP = 8           # gpsimd groups (16 partitions each)
TPG = 33           # 128-atom tiles per group
GRP = TPG * 128    # atoms per group along free dim = 4224
NC_PAD = NGRP * GRP  # padded atoms per core = 33792
GTAB = 1152        # padded graphs per core
CS_W = GRP + 4     # csum row width; col GRP is the zero slot
SCALE = float(1.0 / np.sqrt(F))
EPS = 1e-6
P3W = [1024, 1024, 1024, 1024, 128]   # phase-3 chunk widths per group

_cached = {}
TRACE = False          # set True to capture an NTFF profile
LAST_EXEC_NS = None    # exec_time_ns of the last run when TRACE


def _build_program():
    nc = bacc.Bacc("TRN2", target_bir_lowering=False, debug=False,
                   num_devices=N_CORES)

    # one-hot host-transposed to [128, NC_PAD] fp16 (exact for 0/1 data):
    # feature-major loads are one big contiguous DMA per group and feed the
    # A-matmuls directly (no on-device transpose or convert)
    oh_in = nc.dram_tensor("oh_in", [128, NC_PAD], F16, kind="ExternalInput").ap()
    psi_in = nc.dram_tensor("psi_in", [1, GTAB], F32, kind="ExternalInput").ap()
    gidA_in = nc.dram_tensor("gidA_in", [128, GRP // 16], I16, kind="ExternalInput").ap()
    gidB_in = nc.dram_tensor("gidB_in", [128, GRP // 16], I16, kind="ExternalInput").ap()
    endp_in = nc.dram_tensor("endp_in", [128, GTAB // 16], I16, kind="ExternalInput").ap()
    stap_in = nc.dram_tensor("stap_in", [128, GTAB // 16], I16, kind="ExternalInput").ap()
    sel_in = nc.dram_tensor("sel_in", [128, 2], F32, kind="ExternalInput").ap()
    ident_in = nc.dram_tensor("ident_in", [128, 128], F32, kind="ExternalInput").ap()
    wq_in = nc.dram_tensor("wq_in", [F, E], F32, kind="ExternalInput").ap()
    wk_in = nc.dram_tensor("wk_in", [2, F], F32, kind="ExternalInput").ap()
    wv_in = nc.dram_tensor("wv_in", [2, F], F32, kind="ExternalInput").ap()
    w1_in = nc.dram_tensor("w1_in", [F, F], F32, kind="ExternalInput").ap()
    w2_in = nc.dram_tensor("w2_in", [F, F], F32, kind="ExternalInput").ap()
    out_dram = nc.dram_tensor("out", [NC_PAD, F], F32, kind="ExternalOutput").ap()

    with tile.TileContext(nc) as tc, contextlib.ExitStack() as ctx:
        const = ctx.enter_context(tc.tile_pool(name="const", bufs=1))
        big = ctx.enter_context(tc.tile_pool(name="big", bufs=1))

        # ---- params ----
        wq_s = const.tile([F, E], F32)
        nc.sync.dma_start(wq_s[:], wq_in)
        wv_s = const.tile([2, F], F32)
        nc.sync.dma_start(wv_s[:], wv_in)
        w1_s = const.tile([F, F], F32)
        nc.sync.dma_start(w1_s[:], w1_in)
        w2_s = const.tile([F, F], F32)
        nc.sync.dma_start(w2_s[:], w2_in)
        ident_s = const.tile([128, 128], F32)
        nc.sync.dma_start(ident_s[:], ident_in)
        sel_s = const.tile([128, 2], F32)
        nc.sync.dma_start(sel_s[:], sel_in)
        psi_s = const.tile([1, GTAB], F32)
        nc.sync.dma_start(psi_s[:], psi_in)
        gidA_s = const.tile([128, GRP // 16], I16)
        nc.sync.dma_start(gidA_s[:], gidA_in)
        gidB_s = const.tile([128, GRP // 16], I16)
        nc.sync.dma_start(gidB_s[:], gidB_in)
        endp_s = const.tile([128, GTAB // 16], I16)
        nc.sync.dma_start(endp_s[:], endp_in)
        stap_s = const.tile([128, GTAB // 16], I16)
        nc.sync.dma_start(stap_s[:], stap_in)
        wkT_s = const.tile([F, 2], F32)
        nc.sync.dma_start(wkT_s[:], wk_in.rearrange("a b -> b a"))

        # ---- derived params: u16 = fp16(Wq.T @ Wk.T), W1T, W2T ----
        # phase-3 matmul operands are fp16: fp32 moving data streams at
        # 4 cycles/col on the PE; fp16 streams at 1
        u16 = const.tile([E, 2], F16)
        wv16 = const.tile([2, F], F16)
        nc.scalar.activation(wv16[:], wv_s[:], ACTF.Copy)
        w1t_s = const.tile([F, F], F16)
        w2t_s = const.tile([F, F], F16)
        with tc.tile_pool(name="setup_ps", bufs=1, space="PSUM") as sps:
            u_ps = sps.tile([E, 2], F32)
            nc.tensor.matmul(u_ps[:], lhsT=wq_s[:], rhs=wkT_s[:],
                             start=True, stop=True)
            nc.scalar.activation(u16[:], u_ps[:], ACTF.Copy)
            w1t_ps = sps.tile([F, F], F32)
            nc.tensor.transpose(w1t_ps[:], w1_s[:], ident_s[:])
            nc.scalar.activation(w1t_s[:], w1t_ps[:], ACTF.Copy)
            w2t_ps = sps.tile([F, F], F32)
            nc.tensor.transpose(w2t_ps[:], w2_s[:], ident_s[:])
            nc.scalar.activation(w2t_s[:], w2t_ps[:], ACTF.Copy)

        # ---- phase 1: A (tile-major) -> softplus -> y01 flat fp16 ----
        # y01[16g+r, 128*t+p] = y_{r%2}(atom g*GRP + 128*t + p)
        y01 = big.tile([128, GRP], F16)
        _cached.setdefault("dbg", {})["y01"] = y01.tensor.name
        with tc.tile_pool(name="p1_sbuf", bufs=3) as p1, \
                tc.tile_pool(name="p1_y", bufs=2) as p1y, \
                tc.tile_pool(name="p1_ps", bufs=2, space="PSUM") as p1ps:
            for g in range(NGRP):
                # a01 columns: [0:TPG) = branch 0 by tile, [TPG:2TPG) = branch 1
                a01_ps = p1ps.tile([128, 2 * TPG], F32, tag="a01")
                a01v = a01_ps.rearrange("p (two t) -> p t two", two=2)
                oT16 = p1y.tile([128, GRP], F16, tag="oT")
                nc.sync.dma_start(oT16[:], oh_in[:, g * GRP:(g + 1) * GRP])
                for tl in range(TPG):
                    nc.tensor.matmul(
                        a01v[:, tl:tl + 1, :],
                        lhsT=oT16[0:E, tl * 128:(tl + 1) * 128],
                        rhs=u16[:], start=True, stop=True)
                # softplus(A*s) = max(A,0)*s + ln(1 + exp(-|A|*s))
                # (hw has no softplus table; Exp and Ln share one set)
                absx = p1y.tile([128, 2 * TPG], F32, tag="absx")
                nc.scalar.activation(absx[:], a01_ps[:], ACTF.Abs,
                                     scale=SCALE)
                ex = p1y.tile([128, 2 * TPG], F32, tag="ex")
                nc.scalar.activation(ex[:], absx[:], ACTF.Exp, scale=-1.0)
                lg = p1y.tile([128, 2 * TPG], F32, tag="lg")
                nc.scalar.activation(lg[:], ex[:], ACTF.Ln, bias=1.0)
                rx = p1y.tile([128, 2 * TPG], F32, tag="rx")
                nc.vector.tensor_scalar(rx[:], a01_ps[:], 0.0, SCALE,
                                        ALU.max, ALU.mult)
                ytile = p1y.tile([128, 128], F16, tag="ytile")
                nc.vector.memset(ytile[:, 2 * TPG:128], 0.0)
                nc.vector.tensor_tensor(ytile[:, 0:2 * TPG], rx[:], lg[:],
                                        ALU.add)
                yT = p1y.tile([128, 128], F16, tag="yT")
                nc.sync.dma_start_transpose(yT[:], ytile[:])
                # yT rows [0:TPG) = branch0 tiles, [TPG:2TPG) = branch1.
                # y01 rows per group: [16g:16g+8) = branch0, [16g+8:16g+16) = b1
                for b in range(2):
                    r0 = 16 * g + 8 * b
                    nc.sync.dma_start(
                        y01[r0:r0 + 1, :].rearrange("p (t f) -> p t f", f=128),
                        yT[b * TPG:(b + 1) * TPG, :].rearrange(
                            "t (one f) -> t one f", one=1))
                    nc.sync.dma_start(
                        y01[r0 + 1:r0 + 8, :],
                        y01[r0:r0 + 1, None, :].to_broadcast((1, 7, GRP)))

        # ---- phase 2: cumsum, per-graph denom, f tables, expand ----
        cs = big.tile([128, CS_W], F32)
        _cached.setdefault("dbg", {})["cs"] = cs.tensor.name
        nc.vector.memset(cs[:, GRP:CS_W], 0.0)
        nc.vector.tensor_tensor_scan(cs[:, 0:GRP], y01[:], y01[:], 0.0,
                                     ALU.add, ALU.bypass)
        fyA = big.tile([128, GRP], F32)
        _cached.setdefault("dbg", {})["fyA"] = fyA.tensor.name
        pm = big.tile([128, GRP], F16)
        _cached.setdefault("dbg", {})["pm"] = pm.tensor.name
        with tc.tile_pool(name="p2_sbuf", bufs=1) as p2, \
                tc.tile_pool(name="p2_ps", bufs=1, space="PSUM") as p2ps:
            ge = p2.tile([128, GTAB], F32)
            nc.gpsimd.ap_gather(ge[:], cs[:], endp_s[:], channels=128,
                                num_elems=CS_W, d=1, num_idxs=GTAB)
            gs = p2.tile([128, GTAB], F32)
            nc.gpsimd.ap_gather(gs[:], cs[:], stap_s[:], channels=128,
                                num_elems=CS_W, d=1, num_idxs=GTAB)
            p01 = p2.tile([128, GTAB], F32)
            nc.vector.tensor_tensor(p01[:], ge[:], gs[:], ALU.subtract)
            den_ps = p2ps.tile([2, 1536], F32)
            for c0 in range(0, GTAB, 512):
                c1 = min(c0 + 512, GTAB)
                nc.tensor.matmul(den_ps[:, c0:c1], lhsT=sel_s[:],
                                 rhs=p01[:, c0:c1], start=True, stop=True)
            den01 = p2.tile([2, GTAB], F32)
            nc.scalar.activation(den01[:], den_ps[:, 0:GTAB], ACTF.Copy)
            den0r = den01[0:1, :]
            den1r = p2.tile([1, GTAB], F32)
            nc.sync.dma_start(den1r[:], den01[1:2, :])
            bm = p2.tile([1, GTAB], F32)
            nc.vector.tensor_single_scalar(bm[:], psi_s[:], 0.0, ALU.is_lt)
            # den_sel = den0 + bm*(den1-den0); f = 8*psi/(den_sel + 8*eps)
            nc.vector.tensor_tensor(den1r[:], den1r[:], den0r, ALU.subtract)
            nc.vector.tensor_tensor(den1r[:], den1r[:], bm[:], ALU.mult)
            dsel = p2.tile([1, GTAB], F32)
            nc.vector.tensor_tensor(dsel[:], den0r, den1r[:], ALU.add)
            nc.vector.tensor_single_scalar(dsel[:], dsel[:],
                                           float(NGRP * EPS), ALU.add)
            psi8 = p2.tile([1, GTAB], F32)
            nc.vector.tensor_single_scalar(psi8[:], psi_s[:], float(NGRP),
                                           ALU.mult)
            drec = p2.tile([1, GTAB], F32)
            nc.vector.reciprocal(drec[:], dsel[:])
            f_t = p2.tile([1, GTAB], F32)
            nc.vector.tensor_tensor(f_t[:], psi8[:], drec[:], ALU.mult)
            ftr = p2.tile([1, 2 * GTAB], F32)
            _cached.setdefault("dbg", {})["ftr"] = ftr.tensor.name
            nc.vector.tensor_tensor(ftr[:, GTAB:2 * GTAB], f_t[:], bm[:],
                                    ALU.mult)
            nc.vector.tensor_tensor(ftr[:, 0:GTAB], f_t[:],
                                    ftr[:, GTAB:2 * GTAB], ALU.subtract)
            ftab = p2.tile([128, 2 * GTAB], F32)
            nc.gpsimd.partition_broadcast(ftab[:], ftr[:], channels=128)
            # expand per-graph f to per-atom; odd rows take branch 1
            nc.gpsimd.ap_gather(fyA[:], ftab[:], gidA_s[:], channels=128,
                                num_elems=2 * GTAB, d=1, num_idxs=GRP)
            fyB = p2.tile([128, GRP], F32)
            nc.gpsimd.ap_gather(fyB[:], ftab[:], gidB_s[:], channels=128,
                                num_elems=2 * GTAB, d=1, num_idxs=GRP)
            for g in range(NGRP):
                nc.sync.dma_start(fyA[16 * g + 8:16 * g + 9, :],
                                  fyB[16 * g + 8:16 * g + 9, :])
            nc.vector.tensor_tensor(pm[:], fyA[:], y01[:], ALU.mult)

        # ---- phase 3: v_att, MLP, out (atom-major psum accumulation) ----
        with tc.tile_pool(name="p3_sbuf", bufs=2) as p3, \
                tc.tile_pool(name="p3_out", bufs=3) as p3o, \
                tc.tile_pool(name="p3_vps", bufs=1, space="PSUM") as vps, \
                tc.tile_pool(name="p3_hps", bufs=1, space="PSUM") as hps, \
                tc.tile_pool(name="p3_ops", bufs=1, space="PSUM") as ops:
            for g in range(NGRP):
                col = 0
                for w in P3W:
                    a_off = g * GRP + col
                    # PE operands must be based at partition 0/32/64: stage
                    # the two C rows of this group into partitions 0-1
                    cst = p3.tile([2, w], F16, tag="cst")
                    nc.sync.dma_start(cst[0:1, :],
                                      pm[16 * g:16 * g + 1, col:col + w])
                    nc.sync.dma_start(cst[1:2, :],
                                      pm[16 * g + 8:16 * g + 9, col:col + w])
                    vatt_ps = vps.tile([128, w], F32, tag="vatt")
                    for c0 in range(0, w, 512):
                        cw = min(512, w - c0)
                        nc.tensor.matmul(vatt_ps[:, c0:c0 + cw],
                                         lhsT=wv16[:], rhs=cst[:, c0:c0 + cw],
                                         start=True, stop=True)
                    sv = p3.tile([128, w], F16, tag="sv")
                    nc.scalar.activation(sv[:], vatt_ps[:], ACTF.Silu)
                    h_ps = hps.tile([128, w], F32, tag="h")
                    for c0 in range(0, w, 512):
                        cw = min(512, w - c0)
                        nc.tensor.matmul(h_ps[:, c0:c0 + cw], lhsT=w1t_s[:],
                                         rhs=sv[:, c0:c0 + cw],
                                         start=True, stop=True)
                    sh = p3.tile([128, w], F16, tag="sh")
                    nc.scalar.activation(sh[:], h_ps[:], ACTF.Silu)
                    out_ps = ops.tile([128, w], F32, tag="out")
                    nt = w // 128
                    for t in range(nt):
                        nc.tensor.matmul(
                            out_ps[:, t * 128:(t + 1) * 128],
                            lhsT=cst[:, t * 128:(t + 1) * 128],
                            rhs=wv16[:], start=True, stop=False)
                        nc.tensor.matmul(
                            out_ps[:, t * 128:(t + 1) * 128],
                            lhsT=sh[:, t * 128:(t + 1) * 128],
                            rhs=w2t_s[:], start=False, stop=True)
                    out_sb = p3o.tile([128, w], F32, tag="osb")
                    nc.vector.tensor_copy(out_sb[:], out_ps[:])
                    # store rows permuted (row 8p+t holds atom 128t+p) so
                    # each partition writes one contiguous 4KB run; the host
                    # un-permutes after download
                    nc.sync.dma_start(
                        out_dram[a_off:a_off + w, :].rearrange(
                            "(p t) f -> p t f", p=128),
                        out_sb.rearrange("p (t f) -> p t f", f=F))
                    col += w

    nc.compile()
    return nc


def _get_program():
    if "nc" not in _cached:
        _cached["nc"] = _build_program()
    return _cached["nc"]


def _prepare_core_inputs(oh, psi, bs, Wq, Wk, Wv, W1, W2):
    """Host-side sharding + integer metadata construction."""
    N = oh.shape[0]
    bounds = [0]
    for c in range(1, N_CORES):
        g = bs[(N * c) // N_CORES]
        bounds.append(int(np.searchsorted(bs, g, side="left")))
    bounds.append(N)

    sel = np.zeros((128, 2), np.float32)
    b1rows = (np.arange(128) % 16) >= 8
    sel[~b1rows, 0] = 1.0
    sel[b1rows, 1] = 1.0
    ident = np.eye(128, dtype=np.float32)

    def wrap16(a):
        # a: [NGRP, L] -> [128, L//16] in ap_gather wrapped layout
        L = a.shape[1]
        outw = np.zeros((128, L // 16), a.dtype)
        for g in range(NGRP):
            outw[16 * g:16 * (g + 1), :] = a[g].reshape(L // 16, 16).T
        return outw

    in_maps = []
    meta = []
    for c in range(N_CORES):
        a0, a1 = bounds[c], bounds[c + 1]
        nca = a1 - a0
        assert nca <= NC_PAD, f"core {c} has {nca} atoms > NC_PAD"
        bs_c = bs[a0:a1]
        g0, g1 = int(bs_c[0]), int(bs_c[-1]) + 1
        gtab = g1 - g0
        assert gtab < GTAB, f"core {c} has {gtab} graphs >= GTAB"

        oh_c = np.zeros((128, NC_PAD), np.float16)
        oh_c[:E, :nca] = oh[a0:a1].T  # fp16 is exact for one-hot 0/1 data
        psi_c = np.zeros((1, GTAB), np.float32)
        psi_c[0, :gtab] = psi[g0:g1]

        gid = np.full(NC_PAD, gtab, np.int16)  # pad atoms -> pad graph
        gid[:nca] = (bs_c - g0).astype(np.int16)
        gidA = wrap16(gid.reshape(NGRP, GRP))
        gidB = gidA + np.int16(GTAB)

        gids = np.arange(g0, g1)
        starts = np.searchsorted(bs_c, gids, side="left")
        ends = np.searchsorted(bs_c, gids, side="right")
        endp = np.full((NGRP, GTAB), GRP, np.int16)   # zero slot
        stap = np.full((NGRP, GTAB), GRP, np.int16)
        for g in range(NGRP):
            lo, hi = g * GRP, (g + 1) * GRP
            s_ = np.clip(starts, lo, hi)
            e_ = np.clip(ends, lo, hi)
            has = e_ > s_
            endp[g, :gtab] = np.where(has, e_ - 1 - lo, GRP).astype(np.int16)
            stap[g, :gtab] = np.where(has & (s_ > lo), s_ - 1 - lo,
                                      GRP).astype(np.int16)
        in_maps.append({
            "oh_in": oh_c,
            "psi_in": psi_c,
            "gidA_in": gidA,
            "gidB_in": gidB,
            "endp_in": wrap16(endp),
            "stap_in": wrap16(stap),
            "sel_in": sel,
            "ident_in": ident,
            "wq_in": Wq,
            "wk_in": Wk,
            "wv_in": Wv,
            "w1_in": W1,
            "w2_in": W2,
        })
        meta.append((a0, a1))
    return in_maps, meta


def kernel(elements_one_hot, psi, Wq, Wk, Wv, W1, W2, batch_segments,
           num_graphs):
    oh = np.ascontiguousarray(np.asarray(elements_one_hot, np.float32))
    psi = np.ascontiguousarray(np.asarray(psi, np.float32))
    bs = np.ascontiguousarray(np.asarray(batch_segments, np.int64))
    Wq_ = np.ascontiguousarray(np.asarray(Wq, np.float32))
    Wk_ = np.ascontiguousarray(np.asarray(Wk, np.float32))
    Wv_ = np.ascontiguousarray(np.asarray(Wv, np.float32))
    W1_ = np.ascontiguousarray(np.asarray(W1, np.float32))
    W2_ = np.ascontiguousarray(np.asarray(W2, np.float32))

    in_maps, meta = _prepare_core_inputs(oh, psi, bs, Wq_, Wk_, Wv_, W1_, W2_)
    nc = _get_program()
    global LAST_EXEC_NS
    try:
        res = run_bass_kernel_spmd(nc, in_maps, list(range(N_CORES)),
                                   trace=TRACE)
        LAST_EXEC_NS = res.exec_time_ns
        _cached["last_res"] = res
    except ModuleNotFoundError:
        res = run_bass_kernel_spmd(nc, in_maps, list(range(N_CORES)))
        LAST_EXEC_NS = None
    out = np.zeros((oh.shape[0], F), np.float32)
    for c, (a0, a1) in enumerate(meta):
        dev = np.asarray(res.results[c]["out"])
        # undo the store permutation: per chunk, device row 8p+t is atom 128t+p
        unperm = np.empty_like(dev)
        for g in range(NGRP):
            col = 0
            for w in P3W:
                cb = g * GRP + col
                nt = w // 128
                blk = dev[cb:cb + w].reshape(128, nt, F).transpose(1, 0, 2)
                unperm[cb:cb + w] = blk.reshape(w, F)
                col += w
        out[a0:a1] = unperm[:a1 - a0]
    return out


# revision 34
# speedup vs baseline: 2.8799x; 1.2429x over previous
"""Trainium2 Bass kernel for nn_ChargeSpinEmbedding.

Computation (per atom n with graph g = batch_segments[n]):
    q = onehot @ Wq.T ; k,v = W{k,v}[psi_g < 0]
    y = softplus((q.k)/sqrt(F)) ; att = psi_g * y / (segsum(y) + eps)
    v_att = att * v ; out = v_att + silu(silu(v_att) @ W1.T) @ W2.T

Device algorithm (algebraically identical):
    A_b = onehot @ u_b,  u_b = Wq.T @ Wk[b]   # per-tile matmuls, fp16 inputs
    y_b = softplus(A_b / sqrt(F))             # both branches, select later
    denom_b = segment_sum(y_b)                # cumsum + gather at segment ends
    f = psi / (denom_sel + eps) ; f0 = f*(psi>=0) ; f1 = f*(psi<0)
    C = [f0[g]*y0 ; f1[g]*y1]                 # branch masks pick the right y
    v_att.T = Wv.T @ C                        # feature-major
    out = C.T@Wv + silu(silu(v_att)@W1.T).T @ W2.T   # atom-major psum accum

Sharding: atoms split across 8 cores at graph boundaries (whole graphs live
on one core).  Within a core, atoms are laid out in 8 stripes of GRP=4224
(33 tiles of 128) along the free dim, one stripe per 16-partition group
(GpSimd core granularity for ap_gather).  Per-atom scalar arrays are
[128, GRP] with rows 16g..16g+15 alternating branch0/branch1 values.
"""

import contextlib
import numpy as np

import concourse.bacc as bacc
import concourse.tile as tile
from concourse import mybir
from concourse.bass_utils import run_bass_kernel_spmd

F32 = mybir.dt.float32
F16 = mybir.dt.float16
I16 = mybir.dt.int16
ALU = mybir.AluOpType
ACTF = mybir.ActivationFunctionType

N_CORES = 8
E = 118            # num elements
F = 128            # num features
NGRP = 8           # gpsimd groups (16 partitions each)
TPG = 33           # 128-atom tiles per group
GRP = TPG * 128    # atoms per group along free dim = 4224
NC_PAD = NGRP * GRP  # padded atoms per core = 33792
GTAB = 1152        # padded graphs per core
BLK = 8            # atoms per gather block (graph sizes are >= 8)
NBLK = GRP // BLK  # gather blocks per group = 528
CS_W = GRP + 4     # csum row width; col GRP is the zero slot
SCALE = float(1.0 / np.sqrt(F))
EPS = 1e-6
P3W = [1024, 1024, 1024, 1024, 128]   # phase-3 chunk widths per group

_cached = {}
TRACE = False          # set True to capture an NTFF profile
LAST_EXEC_NS = None    # exec_time_ns of the last run when TRACE


def _build_program():
    nc = bacc.Bacc("TRN2", target_bir_lowering=False, debug=False,
                   num_devices=N_CORES)

    # one-hot host-transposed to [128, NC_PAD] fp16 (exact for 0/1 data):
    # feature-major loads are one big contiguous DMA per group and feed the
    # A-matmuls directly (no on-device transpose or convert)
    oh_in = nc.dram_tensor("oh_in", [128, NC_PAD], F16, kind="ExternalInput").ap()
    psi_in = nc.dram_tensor("psi_in", [1, GTAB], F32, kind="ExternalInput").ap()
    # block-gather metadata: per 8-atom block, the graph at the block start
    # (ixa) and at the block end (ixb); m16 selects per atom between them
    ixa0_in = nc.dram_tensor("ixa0_in", [128, NBLK // 16], I16, kind="ExternalInput").ap()
    ixb0_in = nc.dram_tensor("ixb0_in", [128, NBLK // 16], I16, kind="ExternalInput").ap()
    ixa1_in = nc.dram_tensor("ixa1_in", [128, NBLK // 16], I16, kind="ExternalInput").ap()
    ixb1_in = nc.dram_tensor("ixb1_in", [128, NBLK // 16], I16, kind="ExternalInput").ap()
    m16_in = nc.dram_tensor("m16_in", [128, GRP], F16, kind="ExternalInput").ap()
    endp_in = nc.dram_tensor("endp_in", [128, GTAB // 16], I16, kind="ExternalInput").ap()
    stap_in = nc.dram_tensor("stap_in", [128, GTAB // 16], I16, kind="ExternalInput").ap()
    sel_in = nc.dram_tensor("sel_in", [128, 2], F32, kind="ExternalInput").ap()
    ident_in = nc.dram_tensor("ident_in", [128, 128], F32, kind="ExternalInput").ap()
    wq_in = nc.dram_tensor("wq_in", [F, E], F32, kind="ExternalInput").ap()
    wk_in = nc.dram_tensor("wk_in", [2, F], F32, kind="ExternalInput").ap()
    wv_in = nc.dram_tensor("wv_in", [2, F], F32, kind="ExternalInput").ap()
    w1_in = nc.dram_tensor("w1_in", [F, F], F32, kind="ExternalInput").ap()
    w2_in = nc.dram_tensor("w2_in", [F, F], F32, kind="ExternalInput").ap()
    out_dram = nc.dram_tensor("out", [NC_PAD, F], F32, kind="ExternalOutput").ap()

    with tile.TileContext(nc) as tc, contextlib.ExitStack() as ctx:
        const = ctx.enter_context(tc.tile_pool(name="const", bufs=1))
        big = ctx.enter_context(tc.tile_pool(name="big", bufs=1))

        # ---- params ----
        wq_s = const.tile([F, E], F32)
        nc.sync.dma_start(wq_s[:], wq_in)
        wv_s = const.tile([2, F], F32)
        nc.sync.dma_start(wv_s[:], wv_in)
        w1_s = const.tile([F, F], F32)
        nc.sync.dma_start(w1_s[:], w1_in)
        w2_s = const.tile([F, F], F32)
        nc.sync.dma_start(w2_s[:], w2_in)
        ident_s = const.tile([128, 128], F32)
        nc.sync.dma_start(ident_s[:], ident_in)
        sel_s = const.tile([128, 2], F32)
        nc.sync.dma_start(sel_s[:], sel_in)
        psi_s = const.tile([1, GTAB], F32)
        nc.sync.dma_start(psi_s[:], psi_in)
        ixa0_s = const.tile([128, NBLK // 16], I16)
        nc.sync.dma_start(ixa0_s[:], ixa0_in)
        ixb0_s = const.tile([128, NBLK // 16], I16)
        nc.sync.dma_start(ixb0_s[:], ixb0_in)
        ixa1_s = const.tile([128, NBLK // 16], I16)
        nc.sync.dma_start(ixa1_s[:], ixa1_in)
        ixb1_s = const.tile([128, NBLK // 16], I16)
        nc.sync.dma_start(ixb1_s[:], ixb1_in)
        m16_s = const.tile([128, GRP], F16)
        nc.sync.dma_start(m16_s[:], m16_in)
        endp_s = const.tile([128, GTAB // 16], I16)
        nc.sync.dma_start(endp_s[:], endp_in)
        stap_s = const.tile([128, GTAB // 16], I16)
        nc.sync.dma_start(stap_s[:], stap_in)
        wkT_s = const.tile([F, 2], F32)
        nc.sync.dma_start(wkT_s[:], wk_in.rearrange("a b -> b a"))

        # ---- derived params: u16 = fp16(Wq.T @ Wk.T), W1T, W2T ----
        # phase-3 matmul operands are fp16: fp32 moving data streams at
        # 4 cycles/col on the PE; fp16 streams at 1
        u16 = const.tile([E, 2], F16)
        wv16 = const.tile([2, F], F16)
        nc.scalar.activation(wv16[:], wv_s[:], ACTF.Copy)
        w1t_s = const.tile([F, F], F16)
        w2t_s = const.tile([F, F], F16)
        with tc.tile_pool(name="setup_ps", bufs=1, space="PSUM") as sps:
            u_ps = sps.tile([E, 2], F32)
            nc.tensor.matmul(u_ps[:], lhsT=wq_s[:], rhs=wkT_s[:],
                             start=True, stop=True)
            nc.scalar.activation(u16[:], u_ps[:], ACTF.Copy)
            w1t_ps = sps.tile([F, F], F32)
            nc.tensor.transpose(w1t_ps[:], w1_s[:], ident_s[:])
            nc.scalar.activation(w1t_s[:], w1t_ps[:], ACTF.Copy)
            w2t_ps = sps.tile([F, F], F32)
            nc.tensor.transpose(w2t_ps[:], w2_s[:], ident_s[:])
            nc.scalar.activation(w2t_s[:], w2t_ps[:], ACTF.Copy)

        # ---- phase 1: A (tile-major) -> softplus -> y01 flat fp16 ----
        # y01[16g+r, 128*t+p] = y_{r%2}(atom g*GRP + 128*t + p)
        y01 = big.tile([128, GRP], F16)
        _cached.setdefault("dbg", {})["y01"] = y01.tensor.name
        with tc.tile_pool(name="p1_sbuf", bufs=3) as p1, \
                tc.tile_pool(name="p1_y", bufs=2) as p1y, \
                tc.tile_pool(name="p1_ps", bufs=2, space="PSUM") as p1ps:
            for g in range(NGRP):
                # a01 columns: [0:TPG) = branch 0 by tile, [TPG:2TPG) = branch 1
                a01_ps = p1ps.tile([128, 2 * TPG], F32, tag="a01")
                a01v = a01_ps.rearrange("p (two t) -> p t two", two=2)
                oT16 = p1y.tile([128, GRP], F16, tag="oT")
                nc.sync.dma_start(oT16[:], oh_in[:, g * GRP:(g + 1) * GRP])
                for tl in range(TPG):
                    nc.tensor.matmul(
                        a01v[:, tl:tl + 1, :],
                        lhsT=oT16[0:E, tl * 128:(tl + 1) * 128],
                        rhs=u16[:], start=True, stop=True)
                # softplus(A*s) = max(A,0)*s + ln(1 + exp(-|A|*s))
                # (hw has no softplus table; Exp and Ln share one set)
                absx = p1y.tile([128, 2 * TPG], F32, tag="absx")
                nc.scalar.activation(absx[:], a01_ps[:], ACTF.Abs,
                                     scale=SCALE)
                ex = p1y.tile([128, 2 * TPG], F32, tag="ex")
                nc.scalar.activation(ex[:], absx[:], ACTF.Exp, scale=-1.0)
                lg = p1y.tile([128, 2 * TPG], F32, tag="lg")
                nc.scalar.activation(lg[:], ex[:], ACTF.Ln, bias=1.0)
                rx = p1y.tile([128, 2 * TPG], F32, tag="rx")
                nc.vector.tensor_scalar(rx[:], a01_ps[:], 0.0, SCALE,
                                        ALU.max, ALU.mult)
                ytile = p1y.tile([128, 128], F16, tag="ytile")
                nc.vector.memset(ytile[:, 2 * TPG:128], 0.0)
                nc.vector.tensor_tensor(ytile[:, 0:2 * TPG], rx[:], lg[:],
                                        ALU.add)
                yT = p1y.tile([128, 128], F16, tag="yT")
                nc.sync.dma_start_transpose(yT[:], ytile[:])
                # yT rows [0:TPG) = branch0 tiles, [TPG:2TPG) = branch1.
                # y01 rows per group: [16g:16g+8) = branch0, [16g+8:16g+16) = b1
                for b in range(2):
                    r0 = 16 * g + 8 * b
                    nc.sync.dma_start(
                        y01[r0:r0 + 1, :].rearrange("p (t f) -> p t f", f=128),
                        yT[b * TPG:(b + 1) * TPG, :].rearrange(
                            "t (one f) -> t one f", one=1))
                    nc.sync.dma_start(
                        y01[r0 + 1:r0 + 8, :],
                        y01[r0:r0 + 1, None, :].to_broadcast((1, 7, GRP)))

        # ---- phase 2: cumsum, per-graph denom, f tables, expand ----
        cs = big.tile([128, CS_W], F32)
        _cached.setdefault("dbg", {})["cs"] = cs.tensor.name
        nc.vector.memset(cs[:, GRP:CS_W], 0.0)
        nc.vector.tensor_tensor_scan(cs[:, 0:GRP], y01[:], y01[:], 0.0,
                                     ALU.add, ALU.bypass)
        pmA = big.tile([128, GRP], F16)
        _cached.setdefault("dbg", {})["pmA"] = pmA.tensor.name
        pmB = big.tile([128, GRP], F16)
        _cached.setdefault("dbg", {})["pmB"] = pmB.tensor.name
        with tc.tile_pool(name="p2_sbuf", bufs=1) as p2, \
                tc.tile_pool(name="p2_ps", bufs=1, space="PSUM") as p2ps:
            ge = p2.tile([128, GTAB], F32)
            nc.gpsimd.ap_gather(ge[:], cs[:], endp_s[:], channels=128,
                                num_elems=CS_W, d=1, num_idxs=GTAB)
            gs = p2.tile([128, GTAB], F32)
            nc.gpsimd.ap_gather(gs[:], cs[:], stap_s[:], channels=128,
                                num_elems=CS_W, d=1, num_idxs=GTAB)
            p01 = p2.tile([128, GTAB], F32)
            nc.vector.tensor_tensor(p01[:], ge[:], gs[:], ALU.subtract)
            den_ps = p2ps.tile([2, 1536], F32)
            for c0 in range(0, GTAB, 512):
                c1 = min(c0 + 512, GTAB)
                nc.tensor.matmul(den_ps[:, c0:c1], lhsT=sel_s[:],
                                 rhs=p01[:, c0:c1], start=True, stop=True)
            den01 = p2.tile([2, GTAB], F32)
            nc.scalar.activation(den01[:], den_ps[:, 0:GTAB], ACTF.Copy)
            den0r = den01[0:1, :]
            den1r = p2.tile([1, GTAB], F32)
            nc.sync.dma_start(den1r[:], den01[1:2, :])
            bm = p2.tile([1, GTAB], F32)
            nc.vector.tensor_single_scalar(bm[:], psi_s[:], 0.0, ALU.is_lt)
            # den_sel = den0 + bm*(den1-den0); f = 8*psi/(den_sel + 8*eps)
            nc.vector.tensor_tensor(den1r[:], den1r[:], den0r, ALU.subtract)
            nc.vector.tensor_tensor(den1r[:], den1r[:], bm[:], ALU.mult)
            dsel = p2.tile([1, GTAB], F32)
            nc.vector.tensor_tensor(dsel[:], den0r, den1r[:], ALU.add)
            nc.vector.tensor_single_scalar(dsel[:], dsel[:],
                                           float(NGRP * EPS), ALU.add)
            psi8 = p2.tile([1, GTAB], F32)
            nc.vector.tensor_single_scalar(psi8[:], psi_s[:], float(NGRP),
                                           ALU.mult)
            drec = p2.tile([1, GTAB], F32)
            nc.vector.reciprocal(drec[:], dsel[:])
            f_t = p2.tile([1, GTAB], F32)
            nc.vector.tensor_tensor(f_t[:], psi8[:], drec[:], ALU.mult)
            ftr = p2.tile([1, 2 * GTAB], F32)
            _cached.setdefault("dbg", {})["ftr"] = ftr.tensor.name
            nc.vector.tensor_tensor(ftr[:, GTAB:2 * GTAB], f_t[:], bm[:],
                                    ALU.mult)
            nc.vector.tensor_tensor(ftr[:, 0:GTAB], f_t[:],
                                    ftr[:, GTAB:2 * GTAB], ALU.subtract)
            ftab = p2.tile([128, 2 * GTAB], F32)
            nc.sync.dma_start(
                ftab[:], ftr[0:1, None, :].to_broadcast((1, 128, 2 * GTAB)))
            # expand per-graph f to per-atom in 8-atom blocks: vA = f at the
            # block-start graph, vB = f at the block-end graph, blended by m16
            va0 = p2.tile([128, NBLK], F32)
            nc.gpsimd.ap_gather(va0[:], ftab[:], ixa0_s[:], channels=128,
                                num_elems=2 * GTAB, d=1, num_idxs=NBLK)
            vb0 = p2.tile([128, NBLK], F32)
            nc.gpsimd.ap_gather(vb0[:], ftab[:], ixb0_s[:], channels=128,
                                num_elems=2 * GTAB, d=1, num_idxs=NBLK)
            va1 = p2.tile([128, NBLK], F32)
            nc.gpsimd.ap_gather(va1[:], ftab[:], ixa1_s[:], channels=128,
                                num_elems=2 * GTAB, d=1, num_idxs=NBLK)
            vb1 = p2.tile([128, NBLK], F32)
            nc.gpsimd.ap_gather(vb1[:], ftab[:], ixb1_s[:], channels=128,
                                num_elems=2 * GTAB, d=1, num_idxs=NBLK)
            # pm_b = y01 * (vA_b*(1-m) + vB_b*m); only rows 16g (b=0) and
            # 16g+8 (b=1) are consumed downstream
            ym = p2.tile([128, GRP], F16)
            nc.vector.tensor_tensor(ym[:], y01[:], m16_s[:], ALU.mult)
            ymb = p2.tile([128, GRP], F16)
            nc.vector.tensor_tensor(ymb[:], y01[:], ym[:], ALU.subtract)

            def blend(out_t, va_t, vb_t):
                o3 = out_t.rearrange("p (b i) -> p b i", i=BLK)
                t_ = p2.tile([128, GRP], F16, tag="blend_t", name="t_")
                t3 = t_.rearrange("p (b i) -> p b i", i=BLK)
                u_ = p2.tile([128, GRP], F16, tag="blend_u", name="u_")
                u3 = u_.rearrange("p (b i) -> p b i", i=BLK)
                nc.vector.tensor_tensor(
                    t3, ymb.rearrange("p (b i) -> p b i", i=BLK),
                    va_t[:, :, None].to_broadcast((128, NBLK, BLK)), ALU.mult)
                nc.gpsimd.tensor_tensor(
                    u3, ym.rearrange("p (b i) -> p b i", i=BLK),
                    vb_t[:, :, None].to_broadcast((128, NBLK, BLK)), ALU.mult)
                nc.vector.tensor_tensor(out_t[:], t_[:], u_[:], ALU.add)

            blend(pmA, va0, vb0)
            blend(pmB, va1, vb1)

        # ---- phase 3: v_att, MLP, out (atom-major psum accumulation) ----
        with tc.tile_pool(name="p3_sbuf", bufs=3) as p3, \
                tc.tile_pool(name="p3_out", bufs=3) as p3o, \
                tc.tile_pool(name="p3_vps", bufs=2, space="PSUM") as vps, \
                tc.tile_pool(name="p3_hps", bufs=1, space="PSUM") as hps, \
                tc.tile_pool(name="p3_ops", bufs=1, space="PSUM") as ops:
            # chunk-major over groups: adjacent chunks are independent so the
            # PE never drains while ACT runs the silus of the previous chunk
            cols = [sum(P3W[:i]) for i in range(len(P3W))]
            for (ki, w), g in [((k, w), g) for k, w in enumerate(P3W)
                               for g in range(NGRP)]:
                    col = cols[ki]
                    a_off = g * GRP + col
                    # PE operands must be based at partition 0/32/64: stage
                    # the two C rows of this group into partitions 0-1
                    cst = p3.tile([2, w], F16, tag="cst")
                    nc.sync.dma_start(cst[0:1, :],
                                      pmA[16 * g:16 * g + 1, col:col + w])
                    nc.sync.dma_start(cst[1:2, :],
                                      pmB[16 * g + 8:16 * g + 9, col:col + w])
                    vatt_ps = vps.tile([128, w], F32, tag="vatt")
                    for c0 in range(0, w, 512):
                        cw = min(512, w - c0)
                        nc.tensor.matmul(vatt_ps[:, c0:c0 + cw],
                                         lhsT=wv16[:], rhs=cst[:, c0:c0 + cw],
                                         start=True, stop=True)
                    sv = p3.tile([128, w], F16, tag="sv")
                    nc.scalar.activation(sv[:], vatt_ps[:], ACTF.Silu)
                    h_ps = hps.tile([128, w], F32, tag="h")
                    for c0 in range(0, w, 512):
                        cw = min(512, w - c0)
                        nc.tensor.matmul(h_ps[:, c0:c0 + cw], lhsT=w1t_s[:],
                                         rhs=sv[:, c0:c0 + cw],
                                         start=True, stop=True)
                    sh = p3.tile([128, w], F16, tag="sh")
                    nc.scalar.activation(sh[:], h_ps[:], ACTF.Silu)
                    out_ps = ops.tile([128, w], F32, tag="out")
                    nt = w // 128
                    for t in range(nt):
                        nc.tensor.matmul(
                            out_ps[:, t * 128:(t + 1) * 128],
                            lhsT=cst[:, t * 128:(t + 1) * 128],
                            rhs=wv16[:], start=True, stop=False)
                        nc.tensor.matmul(
                            out_ps[:, t * 128:(t + 1) * 128],
                            lhsT=sh[:, t * 128:(t + 1) * 128],
                            rhs=w2t_s[:], start=False, stop=True)
                    out_sb = p3o.tile([128, w], F32, tag="osb")
                    nc.vector.tensor_copy(out_sb[:], out_ps[:])
                    # store rows permuted (row 8p+t holds atom 128t+p) so
                    # each partition writes one contiguous 4KB run; the host
                    # un-permutes after download
                    nc.sync.dma_start(
                        out_dram[a_off:a_off + w, :].rearrange(
                            "(p t) f -> p t f", p=128),
                        out_sb.rearrange("p (t f) -> p t f", f=F))

    nc.compile()
    return nc


def _get_program():
    if "nc" not in _cached:
        _cached["nc"] = _build_program()
    return _cached["nc"]


def _prepare_core_inputs(oh, psi, bs, Wq, Wk, Wv, W1, W2):
    """Host-side sharding + integer metadata construction."""
    N = oh.shape[0]
    bounds = [0]
    for c in range(1, N_CORES):
        g = bs[(N * c) // N_CORES]
        bounds.append(int(np.searchsorted(bs, g, side="left")))
    bounds.append(N)

    sel = np.zeros((128, 2), np.float32)
    b1rows = (np.arange(128) % 16) >= 8
    sel[~b1rows, 0] = 1.0
    sel[b1rows, 1] = 1.0
    ident = np.eye(128, dtype=np.float32)

    def wrap16(a):
        # a: [NGRP, L] -> [128, L//16] in ap_gather wrapped layout
        L = a.shape[1]
        outw = np.zeros((128, L // 16), a.dtype)
        for g in range(NGRP):
            outw[16 * g:16 * (g + 1), :] = a[g].reshape(L // 16, 16).T
        return outw

    in_maps = []
    meta = []
    for c in range(N_CORES):
        a0, a1 = bounds[c], bounds[c + 1]
        nca = a1 - a0
        assert nca <= NC_PAD, f"core {c} has {nca} atoms > NC_PAD"
        bs_c = bs[a0:a1]
        g0, g1 = int(bs_c[0]), int(bs_c[-1]) + 1
        gtab = g1 - g0
        assert gtab < GTAB, f"core {c} has {gtab} graphs >= GTAB"

        oh_c = np.zeros((128, NC_PAD), np.float16)
        oh_c[:E, :nca] = oh[a0:a1].T  # fp16 is exact for one-hot 0/1 data
        psi_c = np.zeros((1, GTAB), np.float32)
        psi_c[0, :gtab] = psi[g0:g1]

        gid = np.full(NC_PAD, gtab, np.int16)  # pad atoms -> pad graph
        gid[:nca] = (bs_c - g0).astype(np.int16)
        sizes = np.bincount(gid[:nca])
        assert sizes[sizes > 0].min() >= BLK, \
            "block gather needs graph sizes >= BLK"
        g2 = gid.reshape(-1, BLK)
        ixa = g2[:, 0].reshape(NGRP, NBLK)          # graph at block start
        ixb = g2[:, -1].reshape(NGRP, NBLK)         # graph at block end
        m = (gid != np.repeat(g2[:, 0], BLK)).astype(np.float16)
        m16 = np.repeat(m.reshape(NGRP, GRP), 16, axis=0).reshape(128, GRP)
        ixa0 = wrap16(ixa)
        ixb0 = wrap16(ixb)

        gids = np.arange(g0, g1)
        starts = np.searchsorted(bs_c, gids, side="left")
        ends = np.searchsorted(bs_c, gids, side="right")
        endp = np.full((NGRP, GTAB), GRP, np.int16)   # zero slot
        stap = np.full((NGRP, GTAB), GRP, np.int16)
        for g in range(NGRP):
            lo, hi = g * GRP, (g + 1) * GRP
            s_ = np.clip(starts, lo, hi)
            e_ = np.clip(ends, lo, hi)
            has = e_ > s_
            endp[g, :gtab] = np.where(has, e_ - 1 - lo, GRP).astype(np.int16)
            stap[g, :gtab] = np.where(has & (s_ > lo), s_ - 1 - lo,
                                      GRP).astype(np.int16)
        in_maps.append({
            "oh_in": oh_c,
            "psi_in": psi_c,
            "ixa0_in": ixa0,
            "ixb0_in": ixb0,
            "ixa1_in": ixa0 + np.int16(GTAB),
            "ixb1_in": ixb0 + np.int16(GTAB),
            "m16_in": m16,
            "endp_in": wrap16(endp),
            "stap_in": wrap16(stap),
            "sel_in": sel,
            "ident_in": ident,
            "wq_in": Wq,
            "wk_in": Wk,
            "wv_in": Wv,
            "w1_in": W1,
            "w2_in": W2,
        })
        meta.append((a0, a1))
    return in_maps, meta


def kernel(elements_one_hot, psi, Wq, Wk, Wv, W1, W2, batch_segments,
           num_graphs):
    oh = np.ascontiguousarray(np.asarray(elements_one_hot, np.float32))
    psi = np.ascontiguousarray(np.asarray(psi, np.float32))
    bs = np.ascontiguousarray(np.asarray(batch_segments, np.int64))
    Wq_ = np.ascontiguousarray(np.asarray(Wq, np.float32))
    Wk_ = np.ascontiguousarray(np.asarray(Wk, np.float32))
    Wv_ = np.ascontiguousarray(np.asarray(Wv, np.float32))
    W1_ = np.ascontiguousarray(np.asarray(W1, np.float32))
    W2_ = np.ascontiguousarray(np.asarray(W2, np.float32))

    in_maps, meta = _prepare_core_inputs(oh, psi, bs, Wq_, Wk_, Wv_, W1_, W2_)
    nc = _get_program()
    global LAST_EXEC_NS
    try:
        res = run_bass_kernel_spmd(nc, in_maps, list(range(N_CORES)),
                                   trace=TRACE)
        LAST_EXEC_NS = res.exec_time_ns
        _cached["last_res"] = res
    except ModuleNotFoundError:
        res = run_bass_kernel_spmd(nc, in_maps, list(range(N_CORES)))
        LAST_EXEC_NS = None
    out = np.zeros((oh.shape[0], F), np.float32)
    for c, (a0, a1) in enumerate(meta):
        dev = np.asarray(res.results[c]["out"])
        # undo the store permutation: per chunk, device row 8p+t is atom 128t+p
        unperm = np.empty_like(dev)
        for g in range(NGRP):
            col = 0
            for w in P3W:
                cb = g * GRP + col
                nt = w // 128
                blk = dev[cb:cb + w].reshape(128, nt, F).transpose(1, 0, 2)
                unperm[cb:cb + w] = blk.reshape(w, F)
                col += w
        out[a0:a1] = unperm[:a1 - a0]
    return out
